# revision 20
# baseline (speedup 1.0000x reference)
"""Causal self-attention (B=4, T=2048, C=768, 12 heads) on 8 Trainium2 cores.

Sharding: core i handles batch b = i//2 and head-set s = i%2 (6 of 12 heads).
Each core computes x[b] @ W_attn slice -> 6 heads of causal attention -> a
partial projection (row-sharded W_proj).  The host sums the two partials per
batch and adds b_proj.

v2 design (trace-driven rework of the v1 baseline):
  - All matmul operands bf16 (same PE rate as f32r, half the DMA/SBUF).
  - qc-major pipeline: per q-chunk, the V tiles and Q/K projections for that
    chunk are emitted together with the attention tiles, so the tensor
    engine always has independent work while ScalarE streams exp() and the
    HAM clock gate stays warm.
  - Q^T/K^T in pair layout [128, T] (head a on partitions 0-63, b on
    64-127); the two S^T matmuls of a pair run concurrently on the
    row-split PE array.  1/sqrt(64) folded into W_q on host.
  - V' [T, 6*65] with an all-ones column per head (GpSimd memset): the PV
    matmul then yields Y'^T and the softmax denominator row together.
  - exp on ScalarE out of PSUM in [128, 1024] pair tiles with exact causal
    shrink (s0 = 128*m on diagonal tiles); causal masks applied as one
    bf16 multiply per pair tile.
  - Normalize: Y' [65, 512] is copied out of PSUM immediately (frees the
    accumulator bank), then the denominator row is partition-broadcast on
    GpSimd and divided on DVE/GpSimd -- no [1, 512] serial reciprocal.
  - Output projection trails by one q-chunk and fills PE gaps; the result
    staging copies are the only PSUM->SBUF copies left on DVE.
"""

import numpy as np

import concourse.bass as bass
import concourse.mybir as mybir
import concourse.tile as tile
from concourse import bacc

B, T, C = 4, 2048, 768
NH, HD = 12, 64
N_CORES = 8
HPC = 6  # heads per core
P = 128
F32 = mybir.dt.float32
F32R = mybir.dt.float32r
BF16 = mybir.dt.bfloat16
FP8 = mybir.dt.float8e4
QC_N = T // 512  # 4 q-chunks of 512
KC_N = T // P    # 16 k-chunks of 128
CKC = C // P     # 6 contraction chunks for the QKV projection


def build_program(n_iters: int = 1):
    """Builds the SPMD program (identical on all cores; data differs)."""
    nc = bacc.Bacc(
        "TRN2",
        target_bir_lowering=False,
        debug=False,
        enable_asserts=False,
        num_devices=N_CORES,
    )
    d_xt = nc.dram_tensor("xt", [C, T], BF16, kind="ExternalInput").ap()
    d_wq = nc.dram_tensor("wq", [C, 384], BF16, kind="ExternalInput").ap()
    d_wk = nc.dram_tensor("wk", [C, 384], BF16, kind="ExternalInput").ap()
    d_wv = nc.dram_tensor("wv", [C, 390], BF16, kind="ExternalInput").ap()
    d_w2 = nc.dram_tensor("w2", [384, C], BF16, kind="ExternalInput").ap()
    d_masks = nc.dram_tensor("masks", [P, 4 * 1024], BF16, kind="ExternalInput").ap()
    d_ident = nc.dram_tensor("ident", [P, P], F32, kind="ExternalInput").ap()
    d_out = nc.dram_tensor("out", [T, C], F32R, kind="ExternalOutput").ap()

    with tile.TileContext(nc) as tc:
        const_cm = tc.tile_pool(name="const", bufs=1)
        work_cm = tc.tile_pool(name="work", bufs=1)
        sb_cm = tc.tile_pool(name="sbw", bufs=2)
        ps_cm = tc.tile_pool(name="psum", bufs=1, space="PSUM")
        const = const_cm.__enter__()
        work = work_cm.__enter__()
        sbw = sb_cm.__enter__()
        psp = ps_cm.__enter__()

        def proj_chunk(qc, yn_sb, w2_sb):
            """Output projection for q-chunk qc (4 row-blocks of 128)."""
            for qb in range(4 * qc, 4 * qc + 4):
                ob = sbw.tile([P, C], F32R, tag="ob", bufs=4)
                for (n0, nw) in ((0, 512), (512, 256)):
                    po = psp.tile([P, 512], F32, tag="misc", bufs=2,
                                  name=f"po{qb}{n0}")
                    for pp in range(3):
                        nc.tensor.matmul(
                            po[:, :nw],
                            lhsT=yn_sb[pp][:, qb * P:(qb + 1) * P],
                            rhs=w2_sb[pp][:, n0:n0 + nw],
                            start=(pp == 0),
                            stop=(pp == 2),
                        )
                    nc.vector.tensor_copy(ob[:, n0:n0 + nw], po[:, :nw])
                nc.sync.dma_start(d_out[qb * P:(qb + 1) * P, :], ob[:])

        def body(_i=None):
            # ---- persistent tiles ----
            wq_sb = [const.tile([P, 384], BF16, tag=f"wq{k}", name=f"wq{k}") for k in range(CKC)]
            wk_sb = [const.tile([P, 384], BF16, tag=f"wk{k}", name=f"wk{k}") for k in range(CKC)]
            wv_sb = [const.tile([P, 390], BF16, tag=f"wv{k}", name=f"wv{k}") for k in range(CKC)]
            w2_sb = [const.tile([P, C], BF16, tag=f"w2{p}", name=f"w2{p}") for p in range(3)]
            masks_sb = const.tile([P, 4 * 1024], BF16, tag="masks")
            ident_sb = const.tile([P, P], F32, tag="ident")
            xt_sb = [work.tile([P, T], BF16, tag=f"xt{k}", name=f"xt{k}") for k in range(CKC)]
            qt_sb = [work.tile([P, T], BF16, tag=f"qt{p}", name=f"qtp{p}") for p in range(3)]
            kt_sb = [work.tile([P, T], BF16, tag=f"kt{p}", name=f"ktp{p}") for p in range(3)]
            v_sb = [work.tile([P, 390], BF16, tag=f"v{t}", name=f"v{t}") for t in range(KC_N)]
            yn_sb = [work.tile([P, T], BF16, tag=f"yn{p}", name=f"yn{p}") for p in range(3)]

            # ---- loads, in first-use order ----
            for k in range(CKC):
                nc.sync.dma_start(xt_sb[k][:, 0:512], d_xt[k * P:(k + 1) * P, 0:512])
            for k in range(CKC):
                nc.sync.dma_start(wq_sb[k][:], d_wq[k * P:(k + 1) * P, :])
            for k in range(CKC):
                nc.sync.dma_start(wk_sb[k][:], d_wk[k * P:(k + 1) * P, :])
            for k in range(CKC):
                nc.sync.dma_start(wv_sb[k][:], d_wv[k * P:(k + 1) * P, :])
            nc.sync.dma_start(masks_sb[:], d_masks[:])
            nc.sync.dma_start(ident_sb[:], d_ident[:])
            for qq in range(1, QC_N):
                for k in range(CKC):
                    nc.sync.dma_start(
                        xt_sb[k][:, qq * 512:(qq + 1) * 512],
                        d_xt[k * P:(k + 1) * P, qq * 512:(qq + 1) * 512],
                    )
            for p in range(3):
                nc.sync.dma_start(w2_sb[p][:], d_w2[p * P:(p + 1) * P, :])

            def v_tile(t):
                """V' tile for k-chunk t (+ ones column per head)."""
                ps = psp.tile([P, 512], F32, tag="misc", bufs=2,
                              name=f"vps{t}")
                for k in range(CKC):
                    nc.tensor.matmul(
                        ps[:, :390],
                        lhsT=xt_sb[k][:, t * P:(t + 1) * P],
                        rhs=wv_sb[k][:],
                        start=(k == 0),
                        stop=(k == CKC - 1),
                    )
                nc.vector.tensor_copy(v_sb[t][:], ps[:, :390])
                nc.gpsimd.memset(
                    v_sb[t].rearrange("p (h c) -> p h c", h=HPC)[:, :, HD:],
                    1.0,
                )

            def qk_proj(p, qc):
                """Q^T/K^T for pair p, q-chunk qc."""
                for (w_sb, o_sb) in ((wq_sb, qt_sb), (wk_sb, kt_sb)):
                    ps = psp.tile([P, 512], F32, tag="misc", bufs=2,
                                  name=f"qk{qc}{p}")
                    for k in range(CKC):
                        nc.tensor.matmul(
                            ps[:],
                            lhsT=w_sb[k][:, p * P:(p + 1) * P],
                            rhs=xt_sb[k][:, qc * 512:(qc + 1) * 512],
                            start=(k == 0),
                            stop=(k == CKC - 1),
                        )
                    nc.vector.tensor_copy(
                        o_sb[p][:, qc * 512:(qc + 1) * 512], ps[:]
                    )

            def attn_pair(p, qc):
                    # ---- causal attention for (pair p, q-chunk qc) ----
                    n_kc = 4 * qc + 4
                    yps = [psp.tile([P, 512], F32, tag="yp", bufs=2,
                                    name=f"yp{qc}{p}{h2}") for h2 in range(2)]
                    for kc in range(n_kc):
                        m = kc - 4 * qc
                        s0 = 128 * max(m, 0)   # first live q-col in chunk
                        ss = psp.tile([P, 1024], F32, tag="ss", bufs=2)
                        for h2 in range(2):
                            pb = 64 * h2
                            nc.tensor.matmul(
                                ss[:, h2 * 512 + s0:(h2 + 1) * 512],
                                lhsT=kt_sb[p][pb:pb + 64, kc * P:(kc + 1) * P],
                                rhs=qt_sb[p][pb:pb + 64,
                                             qc * 512 + s0:(qc + 1) * 512],
                                start=True,
                                stop=True,
                            )
                        pt = sbw.tile([P, 1024], BF16, tag="pt", bufs=4)
                        if s0:
                            ss_r = ss.rearrange("p (h c) -> p h c", h=2)
                            pt_r = pt.rearrange("p (h c) -> p h c", h=2)
                            nc.scalar.activation(
                                pt_r[:, :, s0:], ss_r[:, :, s0:],
                                mybir.ActivationFunctionType.Exp,
                            )
                        else:
                            nc.scalar.activation(
                                pt[:], ss[:], mybir.ActivationFunctionType.Exp
                            )
                        if m >= 0:
                            # masked multiply: only cols [s0, s0+128) can
                            # violate causality (beyond that all 128 k-rows
                            # are below the diagonal)
                            pt_r = pt.rearrange("p (h c) -> p h c", h=2)
                            mk_r = masks_sb[:, m * 1024:(m + 1) * 1024].rearrange(
                                "p (h c) -> p h c", h=2)
                            nc.vector.tensor_tensor(
                                pt_r[:, :, s0:s0 + 128],
                                pt_r[:, :, s0:s0 + 128],
                                mk_r[:, :, s0:s0 + 128],
                                mybir.AluOpType.mult,
                            )
                        for h2 in range(2):
                            ch = p * 2 + h2
                            nc.tensor.matmul(
                                yps[h2][:65, s0:],
                                lhsT=v_sb[kc][:, ch * 65:(ch + 1) * 65],
                                rhs=pt[:, h2 * 512 + s0:(h2 + 1) * 512],
                                start=(kc == 0),
                                stop=(kc == n_kc - 1),
                            )
                    # ---- normalize: yn = y * (1/d)  (d = row 64) ----
                    # The [1, 512] denominator rows are transposed on the PE
                    # into partition-major [128, 8] so one batched DVE
                    # reciprocal covers the whole (pair, q-chunk), then
                    # transposed back for the broadcast multiply.
                    ysb = sbw.tile([65, 1024], F32, tag="ysb", bufs=2)
                    for h2 in range(2):
                        nc.vector.tensor_copy(
                            ysb[:, h2 * 512:(h2 + 1) * 512], yps[h2][:65, :]
                        )
                    dT = psp.tile([P, 8], F32, tag="misc", bufs=2,
                                  name=f"dT{qc}{p}")
                    for qb in range(4):
                        for h2 in range(2):
                            nc.tensor.transpose(
                                dT[:, 2 * qb + h2:2 * qb + h2 + 1],
                                ysb[64:65,
                                    h2 * 512 + qb * 128:h2 * 512 + (qb + 1) * 128],
                                ident_sb[64:65, 64:65],
                            )
                    rT = sbw.tile([P, 8], F32, tag="rT", bufs=2)
                    with nc.allow_low_precision("f32r is fp32 storage"):
                        nc.vector.reciprocal(rT[:], dT[:])
                    for h2 in range(2):
                        pb = 64 * h2
                        rp = psp.tile([1, 512], F32, tag="misc", bufs=2,
                                      name=f"rp{qc}{p}{h2}")
                        for qb in range(4):
                            nc.tensor.transpose(
                                rp[:, qb * 128:(qb + 1) * 128],
                                rT[:, 2 * qb + h2:2 * qb + h2 + 1],
                                ident_sb[:],
                            )
                        rsb = sbw.tile([1, 512], F32, tag="rsb", bufs=2)
                        nc.vector.tensor_copy(rsb[:], rp[:])
                        dbc = sbw.tile([64, 512], F32, tag="dbc", bufs=2)
                        nc.gpsimd.partition_broadcast(dbc[:], rsb[:])
                        if qc == QC_N - 1 and p == 2:
                            # split so the final projection starts per-block
                            for qb in range(4):
                                nc.vector.tensor_tensor(
                                    yn_sb[p][pb:pb + 64,
                                             qc * 512 + qb * 128:qc * 512 + (qb + 1) * 128],
                                    ysb[:64, h2 * 512 + qb * 128:h2 * 512 + (qb + 1) * 128],
                                    dbc[:, qb * 128:(qb + 1) * 128],
                                    mybir.AluOpType.mult,
                                )
                        else:
                            nc.vector.tensor_tensor(
                                yn_sb[p][pb:pb + 64, qc * 512:(qc + 1) * 512],
                                ysb[:64, h2 * 512:(h2 + 1) * 512],
                                dbc[:],
                                mybir.AluOpType.mult,
                            )

            # Emission order = scheduler priority.  Interleave the pure-PE
            # work (V' for the NEXT q-chunk, Q/K ahead of their pair,
            # trailing projection) between the ACT-bound attention units so
            # the tensor engine never drains at unit boundaries.
            for qc in range(QC_N):
                if qc == 0:
                    qk_proj(0, qc)
                    for t in range(0, 4):
                        v_tile(t)
                    qk_proj(1, qc)
                else:
                    qk_proj(0, qc)
                    qk_proj(1, qc)
                attn_pair(0, qc)
                qk_proj(2, qc)
                if qc + 1 < QC_N:
                    v_tile(4 * qc + 4)
                    v_tile(4 * qc + 5)
                attn_pair(1, qc)
                if qc + 1 < QC_N:
                    v_tile(4 * qc + 6)
                    v_tile(4 * qc + 7)
                if qc > 0:
                    proj_chunk(qc - 1, yn_sb, w2_sb)
                attn_pair(2, qc)
            proj_chunk(QC_N - 1, yn_sb, w2_sb)

        if n_iters == 1:
            body()
        else:
            with tc.For_i(0, n_iters, 1) as _i:
                body(_i)

        for cm in (ps_cm, sb_cm, work_cm, const_cm):
            cm.__exit__(None, None, None)

    nc.compile()
    return nc


def shard_inputs(x, W_attn, b_attn, W_proj, b_proj):
    """Builds the 8 per-core input maps (all host-side numpy prep)."""
    import ml_dtypes

    x = np.asarray(x, dtype=np.float32)
    W_attn = np.asarray(W_attn, dtype=np.float32)
    b_attn = np.asarray(b_attn, dtype=np.float32)
    W_proj = np.asarray(W_proj, dtype=np.float32)
    assert not np.any(b_attn), "v2 kernel assumes zero attention bias"
    scale = float(HD) ** -0.5
    bf16 = ml_dtypes.bfloat16

    kl = np.arange(P)[:, None]
    ql = np.arange(512)[None, :]
    masks = np.concatenate(
        [np.concatenate([(kl <= ql - 128 * m).astype(np.float32)] * 2, axis=1)
         for m in range(4)],
        axis=1,
    ).astype(bf16)  # [128, 4*1024], head-duplicated per m

    in_maps = []
    for core in range(N_CORES):
        b = core // 2
        s = core % 2
        heads = [s * HPC + j for j in range(HPC)]
        xt = np.ascontiguousarray(x[b].T).astype(bf16)  # [C, T]

        wq = np.empty((C, 384), np.float32)
        wk = np.empty((C, 384), np.float32)
        for p in range(3):
            for h2 in range(2):
                hh = heads[p * 2 + h2]
                dst = slice(p * P + h2 * HD, p * P + (h2 + 1) * HD)
                wq[:, dst] = W_attn[:, hh * HD:(hh + 1) * HD] * scale
                wk[:, dst] = W_attn[:, C + hh * HD:C + (hh + 1) * HD]

        wv = np.zeros((C, 390), np.float32)
        for ch in range(HPC):
            hh = heads[ch]
            wv[:, ch * 65:ch * 65 + HD] = (
                W_attn[:, 2 * C + hh * HD:2 * C + (hh + 1) * HD]
            )

        w2 = np.empty((384, C), np.float32)
        for p in range(3):
            for h2 in range(2):
                hh = heads[p * 2 + h2]
                w2[p * P + h2 * HD:p * P + (h2 + 1) * HD, :] = (
                    W_proj[hh * HD:(hh + 1) * HD, :]
                )

        in_maps.append({
            "xt": xt,
            "wq": wq.astype(bf16), "wk": wk.astype(bf16),
            "wv": wv.astype(bf16), "w2": w2.astype(bf16),
            "masks": masks, "ident": np.eye(P, dtype=np.float32),
        })
    return in_maps


def unshard_outputs(results, b_proj):
    b_proj = np.asarray(b_proj, dtype=np.float32)
    out = np.empty((B, T, C), np.float32)
    for b in range(B):
        out[b] = results[2 * b]["out"] + results[2 * b + 1]["out"] + b_proj
    return out


_CACHED_NC = None


def kernel(x, W_attn, b_attn, W_proj, b_proj):
    global _CACHED_NC
    from concourse import bass_utils

    if _CACHED_NC is None:
        _CACHED_NC = build_program(1)
    in_maps = shard_inputs(x, W_attn, b_attn, W_proj, b_proj)
    res = bass_utils.run_bass_kernel_spmd(
        _CACHED_NC, in_maps, core_ids=list(range(N_CORES))
    )
    return unshard_outputs(res.results, b_proj)


# revision 21
# speedup vs baseline: 1.0065x; 1.0065x over previous
"""Causal self-attention (B=4, T=2048, C=768, 12 heads) on 8 Trainium2 cores.

Sharding: core i handles batch b = i//2 and head-set s = i%2 (6 of 12 heads).
Each core computes x[b] @ W_attn slice -> 6 heads of causal attention -> a
partial projection (row-sharded W_proj).  The host sums the two partials per
batch and adds b_proj.

v2 design (trace-driven rework of the v1 baseline):
  - All matmul operands bf16 (same PE rate as f32r, half the DMA/SBUF).
  - qc-major pipeline: per q-chunk, the V tiles and Q/K projections for that
    chunk are emitted together with the attention tiles, so the tensor
    engine always has independent work while ScalarE streams exp() and the
    HAM clock gate stays warm.
  - Q^T/K^T in pair layout [128, T] (head a on partitions 0-63, b on
    64-127); the two S^T matmuls of a pair run concurrently on the
    row-split PE array.  1/sqrt(64) folded into W_q on host.
  - V' [T, 6*65] with an all-ones column per head (GpSimd memset): the PV
    matmul then yields Y'^T and the softmax denominator row together.
  - exp on ScalarE out of PSUM in [128, 1024] pair tiles with exact causal
    shrink (s0 = 128*m on diagonal tiles); causal masks applied as one
    bf16 multiply per pair tile.
  - Normalize: Y' [65, 512] is copied out of PSUM immediately (frees the
    accumulator bank), then the denominator row is partition-broadcast on
    GpSimd and divided on DVE/GpSimd -- no [1, 512] serial reciprocal.
  - Output projection trails by one q-chunk and fills PE gaps; the result
    staging copies are the only PSUM->SBUF copies left on DVE.
"""

import numpy as np

import concourse.bass as bass
import concourse.mybir as mybir
import concourse.tile as tile
from concourse import bacc

B, T, C = 4, 2048, 768
NH, HD = 12, 64
N_CORES = 8
HPC = 6  # heads per core
P = 128
F32 = mybir.dt.float32
F32R = mybir.dt.float32r
BF16 = mybir.dt.bfloat16
FP8 = mybir.dt.float8e4
QC_N = T // 512  # 4 q-chunks of 512
KC_N = T // P    # 16 k-chunks of 128
CKC = C // P     # 6 contraction chunks for the QKV projection


def build_program(n_iters: int = 1):
    """Builds the SPMD program (identical on all cores; data differs)."""
    nc = bacc.Bacc(
        "TRN2",
        target_bir_lowering=False,
        debug=False,
        enable_asserts=False,
        num_devices=N_CORES,
    )
    d_xt = nc.dram_tensor("xt", [C, T], BF16, kind="ExternalInput").ap()
    d_wq = nc.dram_tensor("wq", [C, 384], BF16, kind="ExternalInput").ap()
    d_wk = nc.dram_tensor("wk", [C, 384], BF16, kind="ExternalInput").ap()
    d_wv = nc.dram_tensor("wv", [C, 390], BF16, kind="ExternalInput").ap()
    d_w2 = nc.dram_tensor("w2", [384, C], BF16, kind="ExternalInput").ap()
    d_masks = nc.dram_tensor("masks", [P, 4 * 1024], BF16, kind="ExternalInput").ap()
    d_ident = nc.dram_tensor("ident", [P, P], F32, kind="ExternalInput").ap()
    d_out = nc.dram_tensor("out", [T, C], F32R, kind="ExternalOutput").ap()

    with tile.TileContext(nc) as tc:
        const_cm = tc.tile_pool(name="const", bufs=1)
        work_cm = tc.tile_pool(name="work", bufs=1)
        sb_cm = tc.tile_pool(name="sbw", bufs=2)
        ps_cm = tc.tile_pool(name="psum", bufs=1, space="PSUM")
        const = const_cm.__enter__()
        work = work_cm.__enter__()
        sbw = sb_cm.__enter__()
        psp = ps_cm.__enter__()

        def proj_chunk(qc, yn_sb, w2_sb):
            """Output projection for q-chunk qc (4 row-blocks of 128)."""
            for qb in range(4 * qc, 4 * qc + 4):
                po_hi = psp.tile([P, 512], F32, tag="misc", bufs=2,
                                 name=f"poh{qb}")
                po_lo = psp.tile([P, 512], F32, tag="misc", bufs=2,
                                 name=f"pol{qb}")
                for (tile_, n0, nw) in ((po_hi, 0, 512), (po_lo, 512, 256)):
                    for pp in range(3):
                        nc.tensor.matmul(
                            tile_[:, :nw],
                            lhsT=yn_sb[pp][:, qb * P:(qb + 1) * P],
                            rhs=w2_sb[pp][:, n0:n0 + nw],
                            start=(pp == 0),
                            stop=(pp == 2),
                        )
                ob = sbw.tile([P, C], F32R, tag="ob", bufs=4)
                nc.vector.tensor_copy(ob[:, 0:512], po_hi[:])
                nc.vector.tensor_copy(ob[:, 512:768], po_lo[:, :256])
                nc.sync.dma_start(d_out[qb * P:(qb + 1) * P, :], ob[:])

        def body(_i=None):
            # ---- persistent tiles ----
            wq_sb = [const.tile([P, 384], BF16, tag=f"wq{k}", name=f"wq{k}") for k in range(CKC)]
            wk_sb = [const.tile([P, 384], BF16, tag=f"wk{k}", name=f"wk{k}") for k in range(CKC)]
            wv_sb = [const.tile([P, 390], BF16, tag=f"wv{k}", name=f"wv{k}") for k in range(CKC)]
            w2_sb = [const.tile([P, C], BF16, tag=f"w2{p}", name=f"w2{p}") for p in range(3)]
            masks_sb = const.tile([P, 4 * 1024], BF16, tag="masks")
            ident_sb = const.tile([P, P], F32, tag="ident")
            xt_sb = [work.tile([P, T], BF16, tag=f"xt{k}", name=f"xt{k}") for k in range(CKC)]
            qt_sb = [work.tile([P, T], BF16, tag=f"qt{p}", name=f"qtp{p}") for p in range(3)]
            kt_sb = [work.tile([P, T], BF16, tag=f"kt{p}", name=f"ktp{p}") for p in range(3)]
            v_sb = [work.tile([P, 390], BF16, tag=f"v{t}", name=f"v{t}") for t in range(KC_N)]
            yn_sb = [work.tile([P, T], BF16, tag=f"yn{p}", name=f"yn{p}") for p in range(3)]

            # ---- loads, in first-use order ----
            for k in range(CKC):
                nc.sync.dma_start(xt_sb[k][:, 0:512], d_xt[k * P:(k + 1) * P, 0:512])
                nc.sync.dma_start(wq_sb[k][:], d_wq[k * P:(k + 1) * P, :])
                nc.sync.dma_start(wk_sb[k][:], d_wk[k * P:(k + 1) * P, :])
            for k in range(CKC):
                nc.sync.dma_start(wv_sb[k][:], d_wv[k * P:(k + 1) * P, :])
            nc.sync.dma_start(masks_sb[:], d_masks[:])
            nc.sync.dma_start(ident_sb[:], d_ident[:])
            for qq in range(1, QC_N):
                for k in range(CKC):
                    nc.sync.dma_start(
                        xt_sb[k][:, qq * 512:(qq + 1) * 512],
                        d_xt[k * P:(k + 1) * P, qq * 512:(qq + 1) * 512],
                    )
            for p in range(3):
                nc.sync.dma_start(w2_sb[p][:], d_w2[p * P:(p + 1) * P, :])

            def v_tile(t):
                """V' tile for k-chunk t (+ ones column per head)."""
                ps = psp.tile([P, 512], F32, tag="misc", bufs=2,
                              name=f"vps{t}")
                for k in range(CKC):
                    nc.tensor.matmul(
                        ps[:, :390],
                        lhsT=xt_sb[k][:, t * P:(t + 1) * P],
                        rhs=wv_sb[k][:],
                        start=(k == 0),
                        stop=(k == CKC - 1),
                    )
                nc.vector.tensor_copy(v_sb[t][:], ps[:, :390])
                nc.gpsimd.memset(
                    v_sb[t].rearrange("p (h c) -> p h c", h=HPC)[:, :, HD:],
                    1.0,
                )

            def qk_proj(p, qc):
                """Q^T/K^T for pair p, q-chunk qc."""
                for (w_sb, o_sb) in ((wq_sb, qt_sb), (wk_sb, kt_sb)):
                    ps = psp.tile([P, 512], F32, tag="misc", bufs=2,
                                  name=f"qk{qc}{p}")
                    for k in range(CKC):
                        nc.tensor.matmul(
                            ps[:],
                            lhsT=w_sb[k][:, p * P:(p + 1) * P],
                            rhs=xt_sb[k][:, qc * 512:(qc + 1) * 512],
                            start=(k == 0),
                            stop=(k == CKC - 1),
                        )
                    nc.vector.tensor_copy(
                        o_sb[p][:, qc * 512:(qc + 1) * 512], ps[:]
                    )

            def attn_pair(p, qc):
                    # ---- causal attention for (pair p, q-chunk qc) ----
                    n_kc = 4 * qc + 4
                    yps = [psp.tile([P, 512], F32, tag="yp", bufs=2,
                                    name=f"yp{qc}{p}{h2}") for h2 in range(2)]
                    for kc in range(n_kc):
                        m = kc - 4 * qc
                        s0 = 128 * max(m, 0)   # first live q-col in chunk
                        ss = psp.tile([P, 1024], F32, tag="ss", bufs=2)
                        for h2 in range(2):
                            pb = 64 * h2
                            nc.tensor.matmul(
                                ss[:, h2 * 512 + s0:(h2 + 1) * 512],
                                lhsT=kt_sb[p][pb:pb + 64, kc * P:(kc + 1) * P],
                                rhs=qt_sb[p][pb:pb + 64,
                                             qc * 512 + s0:(qc + 1) * 512],
                                start=True,
                                stop=True,
                            )
                        pt = sbw.tile([P, 1024], BF16, tag="pt", bufs=4)
                        if s0:
                            ss_r = ss.rearrange("p (h c) -> p h c", h=2)
                            pt_r = pt.rearrange("p (h c) -> p h c", h=2)
                            nc.scalar.activation(
                                pt_r[:, :, s0:], ss_r[:, :, s0:],
                                mybir.ActivationFunctionType.Exp,
                            )
                        else:
                            nc.scalar.activation(
                                pt[:], ss[:], mybir.ActivationFunctionType.Exp
                            )
                        if m >= 0:
                            # masked multiply: only cols [s0, s0+128) can
                            # violate causality (beyond that all 128 k-rows
                            # are below the diagonal)
                            pt_r = pt.rearrange("p (h c) -> p h c", h=2)
                            mk_r = masks_sb[:, m * 1024:(m + 1) * 1024].rearrange(
                                "p (h c) -> p h c", h=2)
                            nc.vector.tensor_tensor(
                                pt_r[:, :, s0:s0 + 128],
                                pt_r[:, :, s0:s0 + 128],
                                mk_r[:, :, s0:s0 + 128],
                                mybir.AluOpType.mult,
                            )
                        for h2 in range(2):
                            ch = p * 2 + h2
                            nc.tensor.matmul(
                                yps[h2][:65, s0:],
                                lhsT=v_sb[kc][:, ch * 65:(ch + 1) * 65],
                                rhs=pt[:, h2 * 512 + s0:(h2 + 1) * 512],
                                start=(kc == 0),
                                stop=(kc == n_kc - 1),
                            )
                    # ---- normalize: yn = y * (1/d)  (d = row 64) ----
                    # The [1, 512] denominator rows are transposed on the PE
                    # into partition-major [128, 8] so one batched DVE
                    # reciprocal covers the whole (pair, q-chunk), then
                    # transposed back for the broadcast multiply.
                    ysb = sbw.tile([65, 1024], F32, tag="ysb", bufs=3)
                    for h2 in range(2):
                        nc.vector.tensor_copy(
                            ysb[:, h2 * 512:(h2 + 1) * 512], yps[h2][:65, :]
                        )
                    dT = psp.tile([P, 8], F32, tag="misc", bufs=2,
                                  name=f"dT{qc}{p}")
                    for qb in range(4):
                        for h2 in range(2):
                            nc.tensor.transpose(
                                dT[:, 2 * qb + h2:2 * qb + h2 + 1],
                                ysb[64:65,
                                    h2 * 512 + qb * 128:h2 * 512 + (qb + 1) * 128],
                                ident_sb[64:65, 64:65],
                            )
                    rT = sbw.tile([P, 8], F32, tag="rT", bufs=3)
                    with nc.allow_low_precision("f32r is fp32 storage"):
                        nc.vector.reciprocal(rT[:], dT[:])
                    for h2 in range(2):
                        pb = 64 * h2
                        rp = psp.tile([1, 512], F32, tag="misc", bufs=2,
                                      name=f"rp{qc}{p}{h2}")
                        for qb in range(4):
                            nc.tensor.transpose(
                                rp[:, qb * 128:(qb + 1) * 128],
                                rT[:, 2 * qb + h2:2 * qb + h2 + 1],
                                ident_sb[:],
                            )
                        rsb = sbw.tile([1, 512], F32, tag="rsb", bufs=3)
                        nc.vector.tensor_copy(rsb[:], rp[:])
                        dbc = sbw.tile([64, 512], F32, tag="dbc", bufs=3)
                        nc.gpsimd.partition_broadcast(dbc[:], rsb[:])
                        if qc == QC_N - 1 and p == 2:
                            # split so the final projection starts per-block
                            for qb in range(4):
                                nc.vector.tensor_tensor(
                                    yn_sb[p][pb:pb + 64,
                                             qc * 512 + qb * 128:qc * 512 + (qb + 1) * 128],
                                    ysb[:64, h2 * 512 + qb * 128:h2 * 512 + (qb + 1) * 128],
                                    dbc[:, qb * 128:(qb + 1) * 128],
                                    mybir.AluOpType.mult,
                                )
                        else:
                            nc.vector.tensor_tensor(
                                yn_sb[p][pb:pb + 64, qc * 512:(qc + 1) * 512],
                                ysb[:64, h2 * 512:(h2 + 1) * 512],
                                dbc[:],
                                mybir.AluOpType.mult,
                            )

            # Emission order = scheduler priority.  Interleave the pure-PE
            # work (V' for the NEXT q-chunk, Q/K ahead of their pair,
            # trailing projection) between the ACT-bound attention units so
            # the tensor engine never drains at unit boundaries.
            for qc in range(QC_N):
                if qc == 0:
                    qk_proj(0, qc)
                    for t in range(0, 4):
                        v_tile(t)
                    qk_proj(1, qc)
                else:
                    qk_proj(0, qc)
                    qk_proj(1, qc)
                attn_pair(0, qc)
                qk_proj(2, qc)
                if qc + 1 < QC_N:
                    v_tile(4 * qc + 4)
                    v_tile(4 * qc + 5)
                attn_pair(1, qc)
                if qc + 1 < QC_N:
                    v_tile(4 * qc + 6)
                    v_tile(4 * qc + 7)
                if qc > 0:
                    proj_chunk(qc - 1, yn_sb, w2_sb)
                attn_pair(2, qc)
            proj_chunk(QC_N - 1, yn_sb, w2_sb)

        if n_iters == 1:
            body()
        else:
            with tc.For_i(0, n_iters, 1) as _i:
                body(_i)

        for cm in (ps_cm, sb_cm, work_cm, const_cm):
            cm.__exit__(None, None, None)

    nc.compile()
    return nc


def shard_inputs(x, W_attn, b_attn, W_proj, b_proj):
    """Builds the 8 per-core input maps (all host-side numpy prep)."""
    import ml_dtypes

    x = np.asarray(x, dtype=np.float32)
    W_attn = np.asarray(W_attn, dtype=np.float32)
    b_attn = np.asarray(b_attn, dtype=np.float32)
    W_proj = np.asarray(W_proj, dtype=np.float32)
    assert not np.any(b_attn), "v2 kernel assumes zero attention bias"
    scale = float(HD) ** -0.5
    bf16 = ml_dtypes.bfloat16

    kl = np.arange(P)[:, None]
    ql = np.arange(512)[None, :]
    masks = np.concatenate(
        [np.concatenate([(kl <= ql - 128 * m).astype(np.float32)] * 2, axis=1)
         for m in range(4)],
        axis=1,
    ).astype(bf16)  # [128, 4*1024], head-duplicated per m

    in_maps = []
    for core in range(N_CORES):
        b = core // 2
        s = core % 2
        heads = [s * HPC + j for j in range(HPC)]
        xt = np.ascontiguousarray(x[b].T).astype(bf16)  # [C, T]

        wq = np.empty((C, 384), np.float32)
        wk = np.empty((C, 384), np.float32)
        for p in range(3):
            for h2 in range(2):
                hh = heads[p * 2 + h2]
                dst = slice(p * P + h2 * HD, p * P + (h2 + 1) * HD)
                wq[:, dst] = W_attn[:, hh * HD:(hh + 1) * HD] * scale
                wk[:, dst] = W_attn[:, C + hh * HD:C + (hh + 1) * HD]

        wv = np.zeros((C, 390), np.float32)
        for ch in range(HPC):
            hh = heads[ch]
            wv[:, ch * 65:ch * 65 + HD] = (
                W_attn[:, 2 * C + hh * HD:2 * C + (hh + 1) * HD]
            )

        w2 = np.empty((384, C), np.float32)
        for p in range(3):
            for h2 in range(2):
                hh = heads[p * 2 + h2]
                w2[p * P + h2 * HD:p * P + (h2 + 1) * HD, :] = (
                    W_proj[hh * HD:(hh + 1) * HD, :]
                )

        in_maps.append({
            "xt": xt,
            "wq": wq.astype(bf16), "wk": wk.astype(bf16),
            "wv": wv.astype(bf16), "w2": w2.astype(bf16),
            "masks": masks, "ident": np.eye(P, dtype=np.float32),
        })
    return in_maps


def unshard_outputs(results, b_proj):
    b_proj = np.asarray(b_proj, dtype=np.float32)
    out = np.empty((B, T, C), np.float32)
    for b in range(B):
        out[b] = results[2 * b]["out"] + results[2 * b + 1]["out"] + b_proj
    return out


_CACHED_NC = None


def kernel(x, W_attn, b_attn, W_proj, b_proj):
    global _CACHED_NC
    from concourse import bass_utils

    if _CACHED_NC is None:
        _CACHED_NC = build_program(1)
    in_maps = shard_inputs(x, W_attn, b_attn, W_proj, b_proj)
    res = bass_utils.run_bass_kernel_spmd(
        _CACHED_NC, in_maps, core_ids=list(range(N_CORES))
    )
    return unshard_outputs(res.results, b_proj)
